# revision 21
# baseline (speedup 1.0000x reference)
"""AutoInt (nn_AutoInt_62156766707848) Trainium2 Bass kernel — v2.

Reference math (per sample b of B=2048):
    e   = emb_table[feat_index[b]]            # [F=64, D=128]
    q/k/v/r = e @ W{q,k,v,r}                  # [64, 512] each, split into H=8 heads of P=64
    s_h = q_h @ k_h^T                         # [64, 64]
    att = softmax(s, axis=q)                  # normalize over the QUERY axis
    av  = att @ v_h                           # [64, 64]
    multi = relu(concat_h(av) + e @ Wr)       # [64, 512]
    y   = sigmoid(multi.flatten() @ out_w + out_b)

Sharding: data-parallel over batch; 8 cores x 256 samples. Weights replicated.
Device computes z partials; host applies the final per-sample 64-way sum,
out_b add and sigmoid.

v2 changes over the baseline (654us -> target ~300us):
  - software-pipelined supertiles: stage A (gather/transpose/proj) for st+1 is
    emitted after stage B (attention/output) for st, so every engine queue
    holds ~a full iteration of runnable work and the PE stays dense (HAM warm
    clock: N=512 MM measured 216ns warm vs 427ns cold).
  - v is scaled by 1/Z directly out of PSUM (one tensor_tensor, kills the
    separate PSUM->SBUF v copy).
  - relu(av+r)*w2 fused into one scalar_tensor_tensor per (c, parity) on DVE
    (kills the ACT relu copy and the separate w2 multiply).
  - final reduce over hp is a M=1 ones-matmul on PE accumulating all 4 chunks
    into one PSUM [1, 512] per supertile, shipped straight to DRAM; host sums
    the remaining 64 values per sample.
  - engine rebalance: q/k copies on ACT, eT/Z/recip/vTT/STT on DVE, gathers on
    GPSIMD.

On-core dataflow (per supertile of 8 samples = 512 tokens):
  - indirect-DMA gather of 4x[128,128] bf16 embedding tiles, PE-transpose ->
    eT [128 d, 512 tok]
  - projections on PE: qT/kT chunks [128 hp, 512 tok], v tiles [128 tok, 512 hp]
  - scoresT_h,b [k, 64q] via 64 small K=64 matmuls packed 4-per-PE-pass with
    tile_position; exp on ACT; Z row-sums on DVE; one reciprocal
  - v' = v * 1/Z from PSUM (DVE), avT accumulated on top of rT in PSUM
  - m = relu(av)*w2 (DVE STT), z_c[1, (par,j,q)] += ones^T m_c (PE), DMA out
"""

import sys

sys.path.insert(0, "/opt/trn_rl_repo")

from contextlib import ExitStack

import numpy as np
import ml_dtypes

import concourse.bass as bass
import concourse.tile as tile
from concourse import bacc, mybir
from concourse.bass_utils import run_bass_kernel_spmd
from concourse.masks import make_identity

B, F, D, H, P, V = 2048, 64, 128, 8, 64, 100000
NCORES = 8
ST_SAMPLES = 8                # samples per supertile
TOK = ST_SAMPLES * F          # 512 tokens per supertile

bf16 = mybir.dt.bfloat16
f32 = mybir.dt.float32
i32 = mybir.dt.int32

Exp = mybir.ActivationFunctionType.Exp
X = mybir.AxisListType.X
MUL = mybir.AluOpType.mult
MAX = mybir.AluOpType.max


def build_core_program(bc: int) -> bass.Bass:
    """Build the single-core Bass program for a per-core batch of `bc` samples."""
    assert bc % ST_SAMPLES == 0
    nst = bc // ST_SAMPLES

    nc = bacc.Bacc("TRN2", target_bir_lowering=False, debug=False, num_devices=NCORES)

    # fi is HOST-PERMUTED: fi[p * NG + c] = token_index[c * 128 + p], so a single
    # row-major [128, NG] DMA puts gather c's 128 indices in column c.
    fi = nc.dram_tensor("fi", [bc * F], i32, kind="ExternalInput").ap()
    emb = nc.dram_tensor("emb", [V, D], bf16, kind="ExternalInput").ap()
    wq_d = nc.dram_tensor("wq", [D, H * P], bf16, kind="ExternalInput").ap()
    wk_d = nc.dram_tensor("wk", [D, H * P], bf16, kind="ExternalInput").ap()
    wv_d = nc.dram_tensor("wv", [D, H * P], bf16, kind="ExternalInput").ap()
    wr_d = nc.dram_tensor("wr", [D, H * P], bf16, kind="ExternalInput").ap()
    w2t_d = nc.dram_tensor("w2t", [H * P, F], bf16, kind="ExternalInput").ap()
    # z rows: one per (supertile, chunk); cols (par, j, q); host sums over
    # chunk and q per (par, j)
    zout = nc.dram_tensor("z", [nst, 4, TOK], f32, kind="ExternalOutput").ap()

    with tile.TileContext(nc) as tc:
        with ExitStack() as ctx:
            _body(ctx, tc, nst, fi, emb, (wq_d, wk_d, wv_d, wr_d), w2t_d, zout)
    nc.compile()
    return nc


def _body(ctx, tc, nst, fi, emb, w_drams, w2t_d, zout):
    nc = tc.nc

    cpool = ctx.enter_context(tc.tile_pool(name="const", bufs=1))
    # one slot per gather: gather DMAs must not carry slot-reuse deps (the
    # walrus native path supports a single embedded sync wait per DMA)
    egpool = ctx.enter_context(tc.tile_pool(name="eg", bufs=nst * 4))
    epool = ctx.enter_context(tc.tile_pool(name="et", bufs=2))
    qkpool = ctx.enter_context(tc.tile_pool(name="qk", bufs=8))
    vpool = ctx.enter_context(tc.tile_pool(name="v", bufs=5))
    apool = ctx.enter_context(tc.tile_pool(name="att", bufs=6))
    zpool = ctx.enter_context(tc.tile_pool(name="zr", bufs=2))
    mpool = ctx.enter_context(tc.tile_pool(name="m", bufs=3))

    # PSUM: 8 banks total, one per tag x buf.
    pp_proj = ctx.enter_context(tc.tile_pool(name="pproj", bufs=3, space="PSUM"))
    pp_sc = ctx.enter_context(tc.tile_pool(name="psc", bufs=1, space="PSUM"))
    pp_av = ctx.enter_context(tc.tile_pool(name="pav", bufs=1, space="PSUM"))
    pp_z = ctx.enter_context(tc.tile_pool(name="pz", bufs=1, space="PSUM"))

    # ---- constants
    w_sb = []
    for name, wd in zip(("wq", "wk", "wv", "wr"), w_drams):
        t = cpool.tile([D, H * P], bf16, tag=name + "s", name=name + "s")
        nc.sync.dma_start(out=t[:], in_=wd[:, :])
        w_sb.append(t)
    wq_s, wk_s, wv_s, wr_s = w_sb

    w2t_s = cpool.tile([128, 4 * F], bf16, tag="w2ts")
    nc.sync.dma_start(out=w2t_s[:].rearrange("p (c f) -> p c f", f=F),
                      in_=w2t_d.rearrange("(c p) f -> p c f", p=128))

    ng = nst * 4  # gather count = total tokens / 128
    idx_all = cpool.tile([128, ng], i32, tag="idxall")
    nc.sync.dma_start(out=idx_all[:], in_=fi.rearrange("(a b) -> a b", b=ng))

    ones_sb = cpool.tile([128, 32], bf16, tag="ones")
    nc.gpsimd.memset(ones_sb[:], 1.0)
    warm_sb = cpool.tile([128, 256], bf16, tag="warm")
    nc.gpsimd.memset(warm_sb[:], 0.0)

    # ---- PE warm-up: ~4us of gap-free matmuls flips the HAM clock gate from
    # 1.2 GHz to 2.4 GHz (N=512 MM: 427ns cold -> 216ns warm). The kernel body
    # alone never warms up: its PE stream has frequent dependency micro-gaps.
    # All but the first reuse the loaded weights (ldweights=False).
    warm_ps = pp_z.tile([128, TOK], f32, tag="z", name="warm_ps")
    for i in range(20):
        r = i % 2
        mm = nc.tensor.matmul(out=warm_ps[:, r * 256:(r + 1) * 256],
                              lhsT=warm_sb[:, 0:128], rhs=warm_sb[:],
                              start=True, stop=True, skip_group_check=True)
        if i > 0:
            mm.ldweights = False

    # rolling state between pipeline stages
    state = {}

    def stage_A(st):
        """Gather + transpose + q/k projections for supertile st."""
        e_gs = []
        for g in range(4):
            gg = st * 4 + g
            e_g = egpool.tile([128, 128], bf16, tag="eg")
            nc.gpsimd.indirect_dma_start(
                out=e_g[:], out_offset=None, in_=emb[:, :],
                in_offset=bass.IndirectOffsetOnAxis(ap=idx_all[:, gg:gg + 1], axis=0),
            )
            e_gs.append(e_g)
        eT = epool.tile([128, TOK], bf16, tag="eT")
        for g in range(4):
            nc.sync.dma_start_transpose(out=eT[:, g * 128:(g + 1) * 128],
                                        in_=e_gs[g][:])
        qT, kT = [], []
        for c in range(4):
            psq = pp_proj.tile([128, TOK], f32, tag="proj")
            nc.tensor.matmul(out=psq[:], lhsT=wq_s[:, c * 128:(c + 1) * 128],
                             rhs=eT[:], start=True, stop=True)
            tq = qkpool.tile([128, TOK], bf16, tag="qT")
            nc.vector.tensor_copy(tq[:], psq[:])
            qT.append(tq)
            psk = pp_proj.tile([128, TOK], f32, tag="proj")
            nc.tensor.matmul(out=psk[:], lhsT=wk_s[:, c * 128:(c + 1) * 128],
                             rhs=eT[:], start=True, stop=True)
            tk = qkpool.tile([128, TOK], bf16, tag="kT")
            nc.scalar.copy(tk[:], psk[:])
            kT.append(tk)
        state[st] = {"eT": eT, "qT": qT, "kT": kT}

    def stage_B(s, st_next):
        """Attention + output for supertile s; interleaves stage_A(st_next)
        transposes/projections into the gaps when st_next is not None."""
        ss = state.pop(s)
        eT, qT, kT = ss["eT"], ss["qT"], ss["kT"]

        # ---- scores + exp + Z; v MMs j=0,1 interleaved to fill the EXP-paced
        # gaps on PE (v j>=2 must wait for the 1/Z scaling to free banks)
        att2s = []
        v_ps = []
        zall = zpool.tile([128, 32], bf16, tag="Z")   # cols (c, hh, j)
        for c in range(4):
            sce = pp_sc.tile([128, TOK // 2], f32, tag="sce")
            sco = pp_sc.tile([128, TOK // 2], f32, tag="sco")
            # bank cols: (j, q); rows: (bb, k). hh innermost: consecutive
            # matmuls alternate PE row-groups so LDWEIGHTS pipelines.
            for j in range(4):
                for bb in range(2):
                    b = 2 * j + bb
                    for hh, sc in ((0, sce), (1, sco)):
                        ro = hh * 64
                        nc.tensor.matmul(
                            out=sc[bb * 64:(bb + 1) * 64, j * 64:(j + 1) * 64],
                            lhsT=kT[c][ro:ro + 64, b * 64:(b + 1) * 64],
                            rhs=qT[c][ro:ro + 64, b * 64:(b + 1) * 64],
                            start=True, stop=True,
                            tile_position=(ro, bb * 64),
                        )
            if c < 2:
                ps = pp_proj.tile([128, TOK], f32, tag="proj")
                nc.tensor.matmul(out=ps[:], lhsT=eT[:, c * 128:(c + 1) * 128],
                                 rhs=wv_s[:], start=True, stop=True)
                v_ps.append(ps)
            # att_sb cols: (hh, j, q) — unnormalized exp; 1/Z folded into v
            att_sb = apool.tile([128, TOK], bf16, tag="attsb")
            nc.scalar.activation(out=att_sb[:, :TOK // 2], in_=sce[:], func=Exp)
            nc.scalar.activation(out=att_sb[:, TOK // 2:], in_=sco[:], func=Exp)
            with nc.allow_low_precision(reason="Z in bf16: 0.4% on softmax denom, tol 2e-2"):
                nc.vector.reduce_sum(out=zall[:, c * 8:(c + 1) * 8],
                                     in_=att_sb[:].rearrange("p (g q) -> p g q", q=64), axis=X)
            att2s.append(att_sb)
        zr = zpool.tile([128, 32], f32, tag="Zr")
        nc.vector.reciprocal(zr[:], zall[:])
        zr4 = zr[:].rearrange("p (c hh j) -> p c hh j", hh=2, j=4)

        # ---- v' = v * 1/Z straight out of PSUM (fused scale + downcast)
        vs = []
        for j in range(4):
            if j >= 2:
                ps = pp_proj.tile([128, TOK], f32, tag="proj")
                nc.tensor.matmul(out=ps[:], lhsT=eT[:, j * 128:(j + 1) * 128],
                                 rhs=wv_s[:], start=True, stop=True)
                v_ps.append(ps)
            t = vpool.tile([128, TOK], bf16, tag="vs")
            nc.vector.tensor_tensor(
                out=t[:].rearrange("p (c hh pp) -> p c hh pp", hh=2, pp=64),
                in0=v_ps[j][:].rearrange("p (c hh pp) -> p c hh pp", hh=2, pp=64),
                in1=zr4[:, :, :, j:j + 1].to_broadcast([128, 4, 2, 64]),
                op=MUL,
            )
            vs.append(t)

        # ---- transposes for the next supertile (fills the vTT drain window)
        if st_next is not None:
            e_gs = []
            for g in range(4):
                gg = st_next * 4 + g
                e_g = egpool.tile([128, 128], bf16, tag="eg")
                nc.gpsimd.indirect_dma_start(
                    out=e_g[:], out_offset=None, in_=emb[:, :],
                    in_offset=bass.IndirectOffsetOnAxis(ap=idx_all[:, gg:gg + 1], axis=0),
                )
                e_gs.append(e_g)
            eTn = epool.tile([128, TOK], bf16, tag="eT")
            for g in range(4):
                nc.sync.dma_start_transpose(out=eTn[:, g * 128:(g + 1) * 128],
                                            in_=e_gs[g][:])
            qTn, kTn = [], []

        # ---- rT + avT accumulate; relu*w2 fused; partition-reduce on PE.
        # stage_A(st_next) projections interleaved per-c to keep PE dense
        # while ACT drains the q/k copies.
        eT_par = eT[:].rearrange("p (b2 par q) -> p par b2 q", par=2, q=F)
        # one [128, TOK] PSUM tile (single bank): chunk c's column-sums land on
        # partition rows 32c..32c+31 (M=32 col-tiling; every row of the group
        # holds the same sums) — avoids cross-instruction PSUM accumulation,
        # which silently failed on HW for M=1 matmuls, and non-32-aligned
        # output partitions, which the BIR verifier rejects
        z_ps = pp_z.tile([128, TOK], f32, tag="z")
        for c in range(4):
            ava = pp_av.tile([128, TOK // 2], f32, tag="ava")
            avb = pp_av.tile([128, TOK // 2], f32, tag="avb")
            nc.tensor.matmul(out=ava[:], lhsT=wr_s[:, c * 128:(c + 1) * 128],
                             rhs=eT_par[:, 0:1], start=True, stop=False,
                             skip_group_check=True)
            nc.tensor.matmul(out=avb[:], lhsT=wr_s[:, c * 128:(c + 1) * 128],
                             rhs=eT_par[:, 1:2], start=True, stop=False,
                             skip_group_check=True)
            for hh in range(2):
                for j in range(4):
                    for bb, av in ((0, ava), (1, avb)):
                        nc.tensor.matmul(
                            out=av[hh * 64:(hh + 1) * 64, j * 64:(j + 1) * 64],
                            lhsT=vs[j][bb * 64:(bb + 1) * 64, (2 * c + hh) * 64:(2 * c + hh + 1) * 64],
                            rhs=att2s[c][bb * 64:(bb + 1) * 64, (hh * 4 + j) * 64:(hh * 4 + j + 1) * 64],
                            start=False, stop=True,
                            tile_position=(bb * 64, hh * 64),
                            skip_group_check=True,
                        )
            # m cols: (par, j, q); in-supertile sample b = 2*j + par
            m_sb = mpool.tile([128, TOK], bf16, tag="m")
            w2c = w2t_s[:, c * F:(c + 1) * F]
            for bb, av in ((0, ava), (1, avb)):
                nc.vector.scalar_tensor_tensor(
                    out=m_sb[:, bb * 256:(bb + 1) * 256].rearrange("p (j f) -> p j f", f=F),
                    in0=av[:].rearrange("p (j f) -> p j f", f=F),
                    scalar=0.0,
                    in1=w2c.unsqueeze(1).to_broadcast([128, 4, F]),
                    op0=MAX, op1=MUL,
                )
            # z_ps[32c.., (par, j, q)] = sum_p m_c[p, ...]
            nc.tensor.matmul(out=z_ps[32 * c:32 * (c + 1), :], lhsT=ones_sb[:],
                             rhs=m_sb[:], start=True, stop=True,
                             tile_position=(0, 32 * c), skip_group_check=True)
            # interleave two q/k projections for st_next after each chunk;
            # q drains on DVE, k on ACT to split the copy load
            if st_next is not None:
                for w_s, lst, tag, on_dve in ((wq_s, qTn, "qT", True),
                                              (wk_s, kTn, "kT", False)):
                    ps = pp_proj.tile([128, TOK], f32, tag="proj")
                    nc.tensor.matmul(out=ps[:], lhsT=w_s[:, c * 128:(c + 1) * 128],
                                     rhs=eTn[:], start=True, stop=True)
                    t = qkpool.tile([128, TOK], bf16, tag=tag)
                    if on_dve:
                        nc.vector.tensor_copy(t[:], ps[:])
                    else:
                        nc.scalar.copy(t[:], ps[:])
                    lst.append(t)
        z_sb = zpool.tile([128, TOK], f32, tag="zsb")
        nc.scalar.copy(z_sb[:], z_ps[:])
        nc.sync.dma_start(
            out=zout[s:s + 1, :, :].rearrange("a c t -> (a c) t"),
            in_=z_sb[:].rearrange("(c r) t -> c r t", r=32)[:, 0, :],
        )
        if st_next is not None:
            state[st_next] = {"eT": eTn, "qT": qTn, "kT": kTn}

    stage_A(0)
    for it in range(1, nst + 1):
        stage_B(it - 1, it if it < nst else None)


_NC_CACHE: dict[int, bass.Bass] = {}


def _get_nc(bc: int) -> bass.Bass:
    if bc not in _NC_CACHE:
        _NC_CACHE[bc] = build_core_program(bc)
    return _NC_CACHE[bc]


def z_from_dev(zdev: np.ndarray) -> np.ndarray:
    """Reduce device z [nst, 4, TOK] (cols (par, j, q)) to z [bc] in batch
    order. In-supertile sample b = 2*j + par."""
    nst = zdev.shape[0]
    zi = zdev.reshape(nst, 4, 2, 4, F).sum(axis=(1, 4))   # [nst, par, j]
    return np.ascontiguousarray(zi.transpose(0, 2, 1)).reshape(-1)


def permute_fi(tokens: np.ndarray) -> np.ndarray:
    """Host-side layout for the idx_all tile: fi[p*NG + c] = tokens[c*128 + p]."""
    ng = tokens.shape[0] // 128
    return np.ascontiguousarray(tokens.reshape(ng, 128).T).reshape(-1)


def run_full(feat_index, emb_table, Wq, Wk, Wv, Wr, out_w, out_b, **spmd_kwargs):
    """Shard, run on 8 cores, unshard. Returns (y [B,1] f32, BassKernelResults)."""
    feat_index = np.asarray(feat_index)
    nb = feat_index.shape[0]
    bc = nb // NCORES
    fi = np.stack([
        permute_fi(feat_index.astype(np.int32).reshape(NCORES, bc * F)[i])
        for i in range(NCORES)
    ])
    emb = np.asarray(emb_table, np.float32).astype(ml_dtypes.bfloat16)
    wq = np.asarray(Wq, np.float32).astype(ml_dtypes.bfloat16)
    wk = np.asarray(Wk, np.float32).astype(ml_dtypes.bfloat16)
    wv = np.asarray(Wv, np.float32).astype(ml_dtypes.bfloat16)
    wr = np.asarray(Wr, np.float32).astype(ml_dtypes.bfloat16)
    w2t = np.ascontiguousarray(
        np.asarray(out_w, np.float32).reshape(F, H * P).T
    ).astype(ml_dtypes.bfloat16)

    nc = _get_nc(bc)
    shared = {"emb": emb, "wq": wq, "wk": wk, "wv": wv, "wr": wr, "w2t": w2t}
    in_maps = [{"fi": fi[i], **shared} for i in range(NCORES)]
    res = run_bass_kernel_spmd(nc, in_maps, core_ids=list(range(NCORES)), **spmd_kwargs)
    z = np.concatenate([z_from_dev(r["z"]) for r in res.results])  # [nb]
    z = z + np.float32(np.asarray(out_b, np.float32).reshape(-1)[0])
    y = 1.0 / (1.0 + np.exp(-z, dtype=np.float32))
    return y.reshape(nb, 1).astype(np.float32), res


def kernel(feat_index, emb_table, Wq, Wk, Wv, Wr, out_w, out_b):
    y, _ = run_full(feat_index, emb_table, Wq, Wk, Wv, Wr, out_w, out_b)
    return y
